# revision 24
# baseline (speedup 1.0000x reference)
"""Self-contained Trainium2 Bass kernel for nn_ClassicalGNN_58574763983391.

kernel(**inputs) takes the FULL (unsharded) inputs (as produced by
setup_inputs) and returns the FULL output [512] float32.

Distribution: nodes (and their incident in-edges) are partitioned across
8 NeuronCores on graph boundaries; tiny weights are replicated; node
features are exchanged between layers with an on-device AllGather; the
per-graph readout runs on the core owning the graph.

v2 optimizations over the original staged kernel:
- Batched gathers with the production dma_gather Q7 instruction
  (single_packet=False): one instruction per ~2-12K edges instead of one
  SWDGE generation per 128 edges (~25x fewer Pool-engine instructions).
  Gather tables are bf16 padded to 256B row stride; int16 indices
  require splitting the 102400-row node table into 4 quartile tables,
  so tiles are (superword, quartile)-pure.
- The per-edge lin(eemb[eh]) term is a second batched dma_gather from a
  tiny 17-row table, added chunk-wise on DVE.
- bf16 messages/indicators/layer-matmuls (fp32 PSUM accumulation).
- Superword 128 for indicators (4x less DVE+PE work per edge), with
  z/MLP still done on 512-wide groups.
- Compact bf16 AllGather + local strided expansion into the padded
  gather table (half the exchange bytes of the fp32 version).
"""
import numpy as np

# ----------------------------------------------------------------------
# constants (hardcoded problem shapes)
# ----------------------------------------------------------------------
N = 100000
G = 512
H = 64
NCORES = 8
SW = 128
GRP = 512                  # z/MLP group width (4 superwords)
SHARD = 12800
NTAB = SHARD * NCORES      # 102400
QROWS = NTAB // 4          # 25600 rows per quartile table (int16-safe)
NSW = SHARD // SW          # 100
NET = SHARD // 128         # 100
GMAX = 128
SENT = 255.0               # indicator sentinel for padded edge rows
CH = 6                     # superwords per pipeline chunk


def _bf16(x):
    import ml_dtypes
    return np.asarray(x, np.float32).astype(ml_dtypes.bfloat16)


def _wrap16(vals):
    """dma_gather idx layout: idx i of the linear list sits at
    [i % 16, i // 16], replicated across the 8 Q7 core partition groups."""
    n = vals.shape[0]
    assert n % 16 == 0
    w = vals.reshape(n // 16, 16).T.astype(np.int16)   # [16, n//16]
    return np.tile(w, (8, 1))                          # [128, n//16]


# ----------------------------------------------------------------------
# host-side preprocessing (numpy)
# ----------------------------------------------------------------------
def _shard_graphs(batch):
    cnt = np.bincount(batch, minlength=G)
    cum = np.concatenate([[0], np.cumsum(cnt)])
    bounds_g = [0]
    for c in range(1, NCORES):
        target = N * c / NCORES
        g = int(np.searchsorted(cum, target))
        if g > 0 and target - cum[g - 1] < cum[g] - target:
            g -= 1
        bounds_g.append(max(g, bounds_g[-1]))
    bounds_g.append(G)
    bounds_n = [int(cum[g]) for g in bounds_g]
    assert max(np.diff(bounds_n)) <= SHARD
    return bounds_g, bounds_n


def _build_plans(src, dst, eh, bounds_n):
    core_of = np.searchsorted(np.asarray(bounds_n[1:]), np.arange(N),
                              side="right")
    remap = core_of * SHARD + (np.arange(N) - np.asarray(bounds_n)[core_of])
    raw = []
    counts = np.zeros((NCORES, NSW, 4), np.int64)
    for c in range(NCORES):
        n0, n1 = bounds_n[c], bounds_n[c + 1]
        sel = (dst >= n0) & (dst < n1)
        s, d, e = remap[src[sel]], dst[sel] - n0, eh[sel]
        q = (s // QROWS).astype(np.int64)
        sw = d // SW
        order = np.lexsort((d, q, sw))
        s, d, e, q, sw = s[order], d[order], e[order], q[order], sw[order]
        raw.append(dict(n0=n0, n1=n1, M=n1 - n0, s=s, d=d, e=e, q=q, sw=sw))
        cnt = np.zeros((NSW, 4), np.int64)
        np.add.at(cnt, (sw, q), 1)
        counts[c] = cnt
    tiles_wq = np.maximum((counts.max(axis=0) + 127) // 128, 1)  # [NSW, 4]

    # chunk layout: chunks of CH superwords; within a chunk tiles are
    # ordered q-major so each (chunk, q) is one contiguous gather
    chunks = []      # (w0, w1, tile_base, [per-q tile counts], total_tiles)
    tbase = 0
    for w0 in range(0, NSW, CH):
        w1 = min(w0 + CH, NSW)
        qt = [int(tiles_wq[w0:w1, q].sum()) for q in range(4)]
        chunks.append((w0, w1, tbase, qt, sum(qt)))
        tbase += sum(qt)
    ntiles = tbase

    # slot -> (w, q) map in chunk-major, then q-major, then w order
    slot_w = np.zeros(ntiles, np.int64)
    slot_q = np.zeros(ntiles, np.int64)
    pos = 0
    for (w0, w1, tb, qt, tt) in chunks:
        for q in range(4):
            for w in range(w0, w1):
                k = int(tiles_wq[w, q])
                slot_w[pos:pos + k] = w
                slot_q[pos:pos + k] = q
                pos += k
    assert pos == ntiles

    plans = []
    for c in range(NCORES):
        p = raw[c]
        T = ntiles * 128
        gidx = np.zeros(T, np.int64)
        eidx = np.full(T, 16, np.int64)
        dcol = np.full(T, SENT, np.float32)
        # fill group by group
        tstart = {}
        pos = 0
        for t in range(ntiles):
            key = (slot_w[t], slot_q[t])
            if key not in tstart:
                tstart[key] = t
        for w in range(NSW):
            for q in range(4):
                m = (p["sw"] == w) & (p["q"] == q)
                k = int(m.sum())
                t0 = tstart[(w, q)]
                o = t0 * 128
                gidx[o:o + k] = p["s"][m] - q * QROWS
                eidx[o:o + k] = p["e"][m]
                dcol[o:o + k] = (p["d"][m] - w * SW).astype(np.float32)
        assert gidx.max() < QROWS and gidx.min() >= 0
        plans.append(dict(
            n0=p["n0"], n1=p["n1"], M=p["M"],
            gidx=_wrap16(gidx), eidx=_wrap16(eidx),
            dcols=dcol.reshape(ntiles, 128).T.copy()))
    return plans, chunks, ntiles, slot_w


def _fold_weights(inp, meta):
    emb = [np.asarray(inp[f"emb{i}"], np.float32) for i in range(5)]
    W = np.asarray(inp["proj_w"], np.float32)
    off = [0, 64, 80, 88, 96, 100]
    A = [emb[i] @ W[:, off[i]:off[i + 1]].T for i in range(5)]
    c01 = (A[0][:, None, :] + A[1][None, :, :]).reshape(-1, H) + \
        np.asarray(inp["proj_b"], np.float32)
    c234 = (A[2][:, None, None, :] + A[3][None, :, None, :] +
            A[4][None, None, :, :]).reshape(-1, H)
    c234 = np.concatenate([c234, np.zeros((128 - 70, H), np.float32)])

    t16s = []
    for l in (1, 2, 3):
        t16 = (np.asarray(inp[f"eemb{l}"], np.float32) @
               np.asarray(inp[f"lin_e{l}_w"], np.float32).T +
               np.asarray(inp[f"lin_e{l}_b"], np.float32))
        t = np.zeros((17, 128), np.float32)
        t[:16, :H] = t16
        t16s.append(t)

    def build_blob(entries):
        cols, blob, pos = {}, [], 0
        for name, arr in entries:
            a = np.asarray(arr, np.float32)
            if a.ndim == 1:
                a = a[:, None]
            pad = np.zeros((128, a.shape[1]), np.float32)
            pad[:a.shape[0]] = a
            cols[name] = (pos, a.shape[1], a.shape[0])
            blob.append(pad)
            pos += a.shape[1]
        return cols, np.concatenate(blob, axis=1)

    fentries = [
        ("ident128", np.eye(128, dtype=np.float32)),
        ("rampG", np.tile(np.arange(GMAX, dtype=np.float32), (128, 1))),
        ("gateb", np.full((128, 1),
                          float(np.asarray(inp["gate_b"]).ravel()[0]),
                          np.float32)),
        ("head_w1T", np.asarray(inp["head_w1"], np.float32).T),
        ("head_b1", np.asarray(inp["head_b1"], np.float32)),
        ("head_w2T", np.asarray(inp["head_w2"], np.float32).T),
    ]
    sT = (np.asarray(inp["bn_g"]) /
          np.sqrt(np.asarray(inp["bn_rv"]) + 1e-5)).astype(np.float32)
    bT = (np.asarray(inp["bn_b"]) - np.asarray(inp["bn_rm"]) * sT
          ).astype(np.float32)
    fentries += [("bn_s", sT), ("bn_b", bT)]
    for l in (1, 2, 3):
        fentries += [(f"b1_{l}", np.asarray(inp[f"mlp{l}_b1"], np.float32)),
                     (f"b2_{l}", np.asarray(inp[f"mlp{l}_b2"], np.float32))]
    fcols, fblob = build_blob(fentries)

    bentries = [
        ("bident64", np.eye(64, dtype=np.float32)),
        ("bident128", np.eye(128, dtype=np.float32)),
        ("ramp128", np.tile(np.arange(SW, dtype=np.float32), (128, 1))),
        ("bgatew", np.asarray(inp["gate_w"], np.float32).T),
        ("bonecol", np.ones((128, 1), np.float32)),
        ("bzero64", np.zeros((128, 64), np.float32)),
        ("bramp512", np.tile(np.arange(512, dtype=np.float32), (128, 1))),
    ]
    bentries += [("bbn_s", sT), ("bbn_b", bT)]
    for l in (1, 2, 3):
        bentries += [(f"w1T_{l}", np.asarray(inp[f"mlp{l}_w1"], np.float32).T),
                     (f"w2T_{l}", np.asarray(inp[f"mlp{l}_w2"], np.float32).T)]
    bcols, bblob = build_blob(bentries)

    meta["f_cols"] = fcols
    meta["f_ncols"] = fblob.shape[1]
    meta["b_cols"] = bcols
    meta["b_ncols"] = bblob.shape[1]
    return fblob, _bf16(bblob), _bf16(c01), _bf16(c234), \
        [_bf16(t) for t in t16s]


def _build_inputs(inputs):
    x = np.asarray(inputs["x"], np.int64)
    ei = np.asarray(inputs["edge_index"], np.int64)
    ea = np.asarray(inputs["edge_attr"], np.int64)
    batch = np.asarray(inputs["batch"], np.int64)
    src, dst = ei[0], ei[1]
    eh = (ea[:, 0] + 3 * ea[:, 1] + 7 * ea[:, 2]) % 16

    bounds_g, bounds_n = _shard_graphs(batch)
    plans, chunks, ntiles, slot_w = _build_plans(src, dst, eh, bounds_n)
    meta = dict(chunks=chunks, ntiles=ntiles, slot_w=slot_w,
                eps={l: float(np.asarray(inputs[f"eps{l}"])) for l in (1, 2, 3)},
                head_b2=float(np.asarray(inputs["head_b2"]).ravel()[0]))
    fblob, bblob, c01, c234, t16s = _fold_weights(inputs, meta)

    VOC = [120, 10, 7, 5, 2]
    xi = np.stack([np.clip(x[:, i], 0, VOC[i] - 1) for i in range(5)], 1)
    idx01 = (xi[:, 0] * 10 + xi[:, 1]).astype(np.int32)
    idx234 = (xi[:, 2] * 10 + xi[:, 3] * 2 + xi[:, 4]).astype(np.int32)

    in_maps = []
    for c in range(NCORES):
        p = plans[c]
        n0, n1, M = p["n0"], p["n1"], p["M"]
        i01 = np.zeros(SHARD, np.int32)
        i234 = np.zeros(SHARD, np.int32)
        i01[:M] = idx01[n0:n1]
        i234[:M] = idx234[n0:n1]
        bcol = np.full(SHARD, 600.0, np.float32)
        bcol[:M] = (batch[n0:n1] - bounds_g[c]).astype(np.float32)
        in_maps.append(dict(
            c01=c01, c234=c234,
            t16_1=t16s[0], t16_2=t16s[1], t16_3=t16s[2],
            emb_off0=np.ascontiguousarray(i01.reshape(NET, 128).T),
            emb_off1=np.ascontiguousarray(i234.reshape(NET, 128).T),
            gidx=p["gidx"], eidx=p["eidx"], dcols=p["dcols"],
            fwts=fblob, bwts=bblob,
            bcols=bcol.reshape(NET, 128).T.copy()))
    return in_maps, meta, bounds_g, bounds_n


# ----------------------------------------------------------------------
# device kernel (Bass/Tile)
# ----------------------------------------------------------------------
def _build_kernel(meta):
    import concourse.bass as bass
    import concourse.bacc as bacc
    import concourse.mybir as mybir
    import concourse.tile as tile

    F32 = mybir.dt.float32
    BF16 = mybir.dt.bfloat16
    I32 = mybir.dt.int32
    I16 = mybir.dt.int16
    AF = mybir.ActivationFunctionType
    OP = mybir.AluOpType

    ntiles = int(meta["ntiles"])
    chunks = meta["chunks"]
    slot_w = meta["slot_w"]
    eps = meta["eps"]
    fcols = meta["f_cols"]
    bcols_w = meta["b_cols"]
    FNC = meta["f_ncols"]
    BNC = meta["b_ncols"]
    TKMAX = max(max(tt for (_, _, _, _, tt) in chunks), NET)

    nc = bacc.Bacc("TRN2", target_bir_lowering=False, debug=False,
                   num_devices=NCORES)

    def raw_dma_gather(out_ap, in_ap, idxs_ap, num_idxs, elem_size,
                       elem_step):
        eng = nc.gpsimd
        _in_ap = eng.lower_ap_dma(in_ap, for_custom_bir_dma=True)
        _idxs_ap = eng.lower_ap(idxs_ap)
        _out_ap = eng.lower_ap(out_ap)
        stride_bytes = elem_step * mybir.dt.size(in_ap.dtype)
        assert stride_bytes % 256 == 0
        return eng.add_instruction(
            mybir.InstDMAGatherAnt(
                name=nc.get_next_instruction_name(),
                ins=[*_in_ap, _idxs_ap,
                     eng.lower_val_access(eng.to_reg(num_idxs))],
                outs=[_out_ap],
                transpose=False, num_idxs=num_idxs, elem_size=elem_size,
                stride_bytes_256=stride_bytes // 256,
                gen_mode=0, single_packet=False, queue_num=0,
                sbuf_tokens_per_rank=0, sbuf_free_dim_per_rank=0,
                sbuf_free_dim_pad_per_rank=0, sbuf_byte_offset=0))

    c01 = nc.dram_tensor("c01", [1200, H], BF16, kind="ExternalInput")
    c234 = nc.dram_tensor("c234", [128, H], BF16, kind="ExternalInput")
    t16d = {l: nc.dram_tensor(f"t16_{l}", [17, 128], BF16,
                              kind="ExternalInput") for l in (1, 2, 3)}
    emb_off0 = nc.dram_tensor("emb_off0", [128, NET], I32, kind="ExternalInput")
    emb_off1 = nc.dram_tensor("emb_off1", [128, NET], I32, kind="ExternalInput")
    gidx = nc.dram_tensor("gidx", [128, ntiles * 8], I16, kind="ExternalInput")
    eidx = nc.dram_tensor("eidx", [128, ntiles * 8], I16, kind="ExternalInput")
    dcols = nc.dram_tensor("dcols", [128, ntiles], F32, kind="ExternalInput")
    fwts = nc.dram_tensor("fwts", [128, FNC], F32, kind="ExternalInput")
    bwts = nc.dram_tensor("bwts", [128, BNC], BF16, kind="ExternalInput")
    bcols = nc.dram_tensor("bcols", [128, NET], F32, kind="ExternalInput")
    own_rows = nc.dram_tensor("own_rows", [SHARD, H], BF16)
    h_allc = nc.dram_tensor("h_allc", [NTAB, H], BF16, addr_space="Shared")
    h_q = [nc.dram_tensor(f"h_q{i}", [QROWS, 128], BF16) for i in range(4)]
    out = nc.dram_tensor("out", [1, GMAX], F32, kind="ExternalOutput")
    RG = [list(range(NCORES))]

    with tile.TileContext(nc) as tc:
        with (
            tc.tile_pool(name="const", bufs=1) as cpool,
            tc.tile_pool(name="msg", bufs=2) as msgp,
            tc.tile_pool(name="idxp", bufs=2) as idxp,
            tc.tile_pool(name="ind", bufs=4) as indp,
            tc.tile_pool(name="hT", bufs=1) as hTp,
            tc.tile_pool(name="zz", bufs=3) as zp,
            tc.tile_pool(name="rows", bufs=3) as rowp,
            tc.tile_pool(name="xp", bufs=2) as xp,
            tc.tile_pool(name="ro", bufs=2) as rop,
        ):
            WF = cpool.tile([128, FNC], F32)
            nc.sync.dma_start(WF[:], fwts[:])
            WB = cpool.tile([128, BNC], BF16)
            nc.sync.dma_start(WB[:], bwts[:])

            def fv(name):
                c0, ncol, nrow = fcols[name]
                return WF[0:nrow, c0:c0 + ncol]

            def bv(name):
                c0, ncol, nrow = bcols_w[name]
                return WB[0:nrow, c0:c0 + ncol]

            ident128 = fv("ident128")
            rampG = fv("rampG")
            gatebc = fv("gateb")
            sTc = fv("bn_s")
            bTc = fv("bn_b")
            hw1T = fv("head_w1T")
            hb1c = fv("head_b1")
            hw2T = fv("head_w2T")
            bident64 = bv("bident64")
            bident128 = bv("bident128")
            ramp128 = bv("ramp128")
            bgatew = bv("bgatew")
            bonecol = bv("bonecol")
            bzero64 = bv("bzero64")
            bramp512 = bv("bramp512")
            bbn_s = bv("bbn_s")
            bbn_b = bv("bbn_b")
            mw = {l: dict(w1T=bv(f"w1T_{l}"), w2T=bv(f"w2T_{l}"),
                          b1=fv(f"b1_{l}"), b2=fv(f"b2_{l}"))
                  for l in (1, 2, 3)}

            dcol_sb = cpool.tile([128, ntiles], F32)
            nc.sync.dma_start(dcol_sb[:], dcols[:])
            eoff0_sb = cpool.tile([128, NET], I32)
            nc.sync.dma_start(eoff0_sb[:], emb_off0[:])
            eoff1_sb = cpool.tile([128, NET], I32)
            nc.sync.dma_start(eoff1_sb[:], emb_off1[:])
            bcol_sb = cpool.tile([128, NET], F32)
            nc.sync.dma_start(bcol_sb[:], bcols[:])
            hbnrows = cpool.tile([128, NET * H], BF16)

            hTa = hTp.tile([64, SHARD], BF16, tag="hTa")
            hTb = hTp.tile([64, SHARD], BF16, tag="hTb")

            def expand_h_all():
                # h_allc [NTAB, 64] -> h_all [NTAB, 128] (pad cols 64:128)
                for b0 in range(0, NTAB, 6400):
                    xt = xp.tile([128, 50 * H], BF16, tag="xt")
                    nc.sync.dma_start(
                        xt[:].rearrange("p (c h) -> p c h", h=H),
                        h_allc[b0:b0 + 6400, :].rearrange(
                            "(c p) h -> p c h", p=128))
                    nc.sync.dma_start(
                        h_q[b0 // QROWS][b0 % QROWS:b0 % QROWS + 6400, 0:H]
                        .rearrange("(c p) h -> p c h", p=128),
                        xt[:].rearrange("p (c h) -> p c h", h=H))

            with (
                tc.tile_pool(name="pa", bufs=4, space="PSUM") as pap,
                tc.tile_pool(name="ps", bufs=2, space="PSUM") as ps,
            ):
                # ---- embed (per-tile indirect gathers, known-good) ----
                assert TKMAX >= NET
                eslab = msgp.tile([128, TKMAX * H], BF16, tag="mslab")
                for t in range(NET):
                    sl = eslab[:, t * H:(t + 1) * H]
                    nc.gpsimd.indirect_dma_start(
                        out=sl, out_offset=None, in_=c01[:],
                        in_offset=bass.IndirectOffsetOnAxis(
                            ap=eoff0_sb[:, t:t + 1], axis=0))
                    nc.gpsimd.indirect_dma_start(
                        out=sl, out_offset=None, in_=c234[:],
                        in_offset=bass.IndirectOffsetOnAxis(
                            ap=eoff1_sb[:, t:t + 1], axis=0),
                        compute_op=OP.add)
                nc.sync.dma_start(
                    own_rows[:].rearrange("(c p) h -> p c h", p=128),
                    eslab[:, :NET * H].rearrange("p (c h) -> p c h", h=H))
                for t in range(NET):
                    pt = ps.tile([128, 128], BF16, tag="trx")
                    nc.tensor.transpose(
                        out=pt[0:64, :], in_=eslab[:, t * H:(t + 1) * H],
                        identity=bident128[:])
                    nc.scalar.activation(hTa[:, 128 * t:128 * (t + 1)],
                                         pt[0:64, :], AF.Copy)
                nc.gpsimd.collective_compute(
                    "AllGather", OP.bypass, replica_groups=RG,
                    ins=[own_rows[:]], outs=[h_allc[:]])
                expand_h_all()

                # ---- layers ----
                cur, nxt = hTa, hTb
                for l in (1, 2, 3):
                    for (w0, w1, tbase, qt, tt) in chunks:
                        mslab = msgp.tile([128, TKMAX * H], BF16, tag="mslab")
                        tslab = msgp.tile([128, TKMAX * H], BF16, tag="tslab")
                        gsl = idxp.tile([128, TKMAX * 8], I16, tag="gsl")
                        nc.sync.dma_start(gsl[:, :tt * 8],
                                          gidx[:, tbase * 8:(tbase + tt) * 8])
                        esl = idxp.tile([128, TKMAX * 8], I16, tag="esl")
                        nc.sync.dma_start(esl[:, :tt * 8],
                                          eidx[:, tbase * 8:(tbase + tt) * 8])
                        # per-quartile h gathers
                        o = 0
                        for q in range(4):
                            k = qt[q]
                            if k == 0:
                                continue
                            raw_dma_gather(
                                mslab[:, o * H:(o + k) * H].rearrange(
                                    "p (k h) -> p k h", h=H),
                                h_q[q][:],
                                gsl[:, o * 8:(o + k) * 8],
                                k * 128, H, 128)
                            o += k
                        # t16 gather (single instruction per chunk)
                        raw_dma_gather(
                            tslab[:, :tt * H].rearrange(
                                "p (k h) -> p k h", h=H),
                            t16d[l][:], esl[:, :tt * 8], tt * 128, H, 128)
                        nc.vector.tensor_tensor(
                            out=mslab[:, :tt * H], in0=mslab[:, :tt * H],
                            in1=tslab[:, :tt * H], op=OP.add)
                        nc.scalar.activation(mslab[:, :tt * H],
                                             mslab[:, :tt * H], AF.Relu)
                        # scatter: per-tile indicator matmuls into group PSUM
                        ngrp = (w1 - w0 + 3) // 4
                        pas = []
                        glast = {}
                        for j in range(tt):
                            g = (int(slot_w[tbase + j]) - w0) // 4
                            glast[g] = j
                        for _g in range(ngrp):
                            gw0 = w0 + _g * 4
                            gcols = min(4, w1 - gw0) * SW
                            pa_t = pap.tile([64, GRP], F32, tag="agg")
                            pas.append(pa_t)
                            nc.tensor.matmul(
                                out=pa_t[:, :gcols], lhsT=bzero64[:],
                                rhs=bramp512[:, :gcols], start=True,
                                stop=False, skip_group_check=True)
                        for j in range(tt):
                            t = tbase + j
                            w = int(slot_w[t])
                            g = (w - w0) // 4
                            col = (w - w0) % 4
                            ind = indp.tile([128, SW], BF16, tag="ind")
                            nc.vector.tensor_scalar(
                                out=ind[:], in0=ramp128[:],
                                scalar1=dcol_sb[:, t:t + 1], scalar2=None,
                                op0=OP.is_equal)
                            nc.tensor.matmul(
                                out=pas[g][:, col * SW:(col + 1) * SW],
                                lhsT=mslab[:, j * H:(j + 1) * H],
                                rhs=ind[:], start=False,
                                stop=(glast[g] == j),
                                skip_group_check=True)
                        # z + MLP on 512-wide groups
                        for g in range(ngrp):
                            gw0 = w0 + g * 4
                            gcols = min(4, w1 - gw0) * SW
                            sl = slice(gw0 * SW, gw0 * SW + gcols)
                            aggb = zp.tile([64, GRP], BF16, tag="aggb")
                            nc.scalar.activation(aggb[:, :gcols],
                                                 pas[g][:, :gcols], AF.Copy)
                            zt = zp.tile([64, GRP], BF16, tag="zt")
                            nc.vector.scalar_tensor_tensor(
                                out=zt[:, :gcols], in0=cur[:, sl],
                                scalar=1.0 + eps[l],
                                in1=aggb[:, :gcols], op0=OP.mult, op1=OP.add)
                            py1 = ps.tile([64, GRP], F32, tag="py")
                            nc.tensor.matmul(out=py1[:, :gcols],
                                             lhsT=mw[l]["w1T"],
                                             rhs=zt[:, :gcols],
                                             start=True, stop=True,
                                             skip_group_check=True)
                            y1 = zp.tile([64, GRP], BF16, tag="y1")
                            nc.scalar.activation(y1[:, :gcols],
                                                 py1[:, :gcols], AF.Relu,
                                                 bias=mw[l]["b1"])
                            py2 = ps.tile([64, GRP], F32, tag="py")
                            nc.tensor.matmul(out=py2[:, :gcols],
                                             lhsT=mw[l]["w2T"],
                                             rhs=y1[:, :gcols],
                                             start=True, stop=True,
                                             skip_group_check=True)
                            nc.scalar.activation(nxt[:, sl], py2[:, :gcols],
                                                 AF.Relu, bias=mw[l]["b2"])
                    if l < 3:
                        for c0 in range(0, NET, 4):
                            c1 = min(c0 + 4, NET)
                            stage = rowp.tile([128, 4 * H], BF16, tag="stage")
                            for t in range(c0, c1):
                                pt = ps.tile([128, 128], BF16, tag="trx")
                                nc.tensor.transpose(
                                    out=pt[:, 0:64],
                                    in_=nxt[:, 128 * t:128 * (t + 1)],
                                    identity=bident64[:])
                                nc.scalar.activation(
                                    stage[:, (t - c0) * H:(t - c0 + 1) * H],
                                    pt[:, 0:64], AF.Copy)
                            nc.sync.dma_start(
                                own_rows[128 * c0:128 * c1, :].rearrange(
                                    "(c p) h -> p c h", p=128),
                                stage[:, :(c1 - c0) * H].rearrange(
                                    "p (c h) -> p c h", h=H))
                        nc.gpsimd.collective_compute(
                            "AllGather", OP.bypass, replica_groups=RG,
                            ins=[own_rows[:]], outs=[h_allc[:]])
                        expand_h_all()
                    cur, nxt = nxt, cur

            # ---- readout ----
            with (
                tc.tile_pool(name="psra", bufs=2, space="PSUM") as psra,
                tc.tile_pool(name="psrb", bufs=1, space="PSUM") as psrb,
            ):
                hbnT = hTp.tile([64, SHARD], BF16, tag="hbnT")
                nc.vector.scalar_tensor_tensor(
                    out=hbnT[:], in0=cur[:, :SHARD], scalar=bbn_s[:, 0:1],
                    in1=bbn_b[:, 0:1].to_broadcast([64, SHARD]),
                    op0=OP.mult, op1=OP.add)
                pg = psrb.tile([128, NET], F32, tag="aggg")
                for t in range(NET):
                    pt = psra.tile([128, 64], BF16, tag="trb2")
                    nc.tensor.transpose(out=pt[:],
                                        in_=hbnT[:, 128 * t:128 * (t + 1)],
                                        identity=bident64[:])
                    nc.scalar.activation(hbnrows[:, t * H:(t + 1) * H], pt[:],
                                         AF.Copy)
                    nc.tensor.matmul(out=pg[:, t:t + 1],
                                     lhsT=hbnT[:, 128 * t:128 * (t + 1)],
                                     rhs=bgatew[:], start=True, stop=True,
                                     skip_group_check=True)
                wsb = rop.tile([128, NET], F32, tag="wsb")
                nc.scalar.activation(wsb[:], pg[:], AF.Exp,
                                     bias=gatebc[:, 0:1])
                pgv = psrb.tile([128, H], F32, tag="pgv")
                pden = psrb.tile([128, 1], F32, tag="pden")
                for t in range(NET):
                    wind = indp.tile([128, GMAX], BF16, tag="wind")
                    nc.vector.scalar_tensor_tensor(
                        out=wind[:], in0=rampG[:], scalar=bcol_sb[:, t:t + 1],
                        in1=wsb[:, t:t + 1].to_broadcast([128, GMAX]),
                        op0=OP.is_equal, op1=OP.mult)
                    nc.tensor.matmul(out=pgv[:], lhsT=wind[:],
                                     rhs=hbnrows[:, t * H:(t + 1) * H],
                                     start=(t == 0), stop=(t == NET - 1),
                                     skip_group_check=True)
                    nc.tensor.matmul(out=pden[:], lhsT=wind[:], rhs=bonecol[:],
                                     start=(t == 0), stop=(t == NET - 1),
                                     skip_group_check=True)
                rden = rop.tile([128, 1], F32, tag="rden")
                nc.vector.reciprocal(rden[:], pden[:])
                gv = rop.tile([128, H], F32, tag="gv")
                nc.scalar.activation(gv[:], pgv[:], AF.Copy,
                                     scale=rden[:, 0:1])
                ptg = psrb.tile([64, 128], F32, tag="tr")
                nc.tensor.transpose(out=ptg[:], in_=gv[:],
                                    identity=ident128[:])
                gvT = rop.tile([64, 128], F32, tag="gvT")
                nc.scalar.activation(gvT[:], ptg[:], AF.Copy)
                ph1 = psrb.tile([128, GMAX], F32, tag="ph1")
                nc.tensor.matmul(out=ph1[:], lhsT=hw1T[:], rhs=gvT[:],
                                 start=True, stop=True, skip_group_check=True)
                y1r = rop.tile([128, GMAX], F32, tag="y1r")
                nc.scalar.activation(y1r[:], ph1[:], AF.Relu,
                                     bias=hb1c[:, 0:1])
                po = psrb.tile([1, GMAX], F32, tag="po")
                nc.tensor.matmul(out=po[:], lhsT=hw2T[:], rhs=y1r[:],
                                 start=True, stop=True, skip_group_check=True)
                osb = rop.tile([1, GMAX], F32, tag="osb")
                nc.scalar.activation(osb[:], po[:], AF.Copy,
                                     bias=float(meta["head_b2"]))
                nc.sync.dma_start(out[:], osb[:])

    nc.compile()
    return nc


# ----------------------------------------------------------------------
# runner (bass2jax/PJRT shard_map over 8 cores)
# ----------------------------------------------------------------------
def _make_runner(nc, n_cores=NCORES):
    import jax
    from jax.sharding import Mesh, PartitionSpec
    from jax.experimental.shard_map import shard_map
    import concourse.mybir as mybir
    from concourse.bass2jax import (_bass_exec_p, install_neuronx_cc_hook,
                                    partition_id_tensor)

    install_neuronx_cc_hook()
    partition_name = (nc.partition_id_tensor.name
                      if nc.partition_id_tensor else None)
    in_names, out_names, out_avals, zero_outs = [], [], [], []
    for alloc in nc.m.functions[0].allocations:
        if not isinstance(alloc, mybir.MemoryLocationSet):
            continue
        name = alloc.memorylocations[0].name
        if alloc.kind == "ExternalInput":
            if name != partition_name:
                in_names.append(name)
        elif alloc.kind == "ExternalOutput":
            out_names.append(name)
            shape = tuple(alloc.tensor_shape)
            dtype = mybir.dt.np(alloc.dtype)
            out_avals.append(jax.core.ShapedArray(shape, dtype))
            zero_outs.append(np.zeros(shape, dtype))
    n_params = len(in_names)
    n_outs = len(out_avals)
    all_in = list(in_names) + list(out_names)
    if partition_name is not None:
        all_in.append(partition_name)

    def _body(*args):
        operands = list(args)
        if partition_name is not None:
            operands.append(partition_id_tensor())
        return tuple(_bass_exec_p.bind(
            *operands, out_avals=tuple(out_avals), in_names=tuple(all_in),
            out_names=tuple(out_names), lowering_input_output_aliases=(),
            sim_require_finite=True, sim_require_nnan=True, nc=nc))

    devices = jax.devices()[:n_cores]
    mesh = Mesh(np.asarray(devices), ("core",))
    in_specs = (PartitionSpec("core"),) * (n_params + n_outs)
    out_specs = (PartitionSpec("core"),) * len(out_names)
    donate = tuple(range(n_params, n_params + n_outs))
    fn = jax.jit(shard_map(_body, mesh=mesh, in_specs=in_specs,
                           out_specs=out_specs, check_rep=False),
                 donate_argnums=donate, keep_unused=True)

    def pack(in_maps):
        per_core = [[np.asarray(m[nm]) for nm in in_names] for m in in_maps]
        return [np.concatenate([per_core[c][i] for c in range(n_cores)],
                               axis=0) for i in range(n_params)]

    def zpack():
        return [np.concatenate([z] * n_cores, axis=0) for z in zero_outs]

    def run(in_maps):
        import jax as _jax
        outs = fn(*pack(in_maps), *zpack())
        _jax.block_until_ready(outs)
        res = []
        for c in range(n_cores):
            m = {}
            for i, nm in enumerate(out_names):
                arr = np.asarray(outs[i])
                per = arr.shape[0] // n_cores
                m[nm] = arr[c * per:(c + 1) * per]
            res.append(m)
        return res

    def timeit(in_maps, iters=8, warmup=2):
        import time as _time
        import jax as _jax
        from jax.sharding import NamedSharding
        sh = NamedSharding(mesh, PartitionSpec("core"))
        args = [_jax.device_put(a, sh) for a in pack(in_maps)]
        zos = [[_jax.device_put(z, sh) for z in zpack()]
               for _ in range(iters + warmup)]
        for i in range(warmup):
            _jax.block_until_ready(fn(*args, *zos[i]))
        ts = []
        for i in range(iters):
            t0 = _time.perf_counter()
            _jax.block_until_ready(fn(*args, *zos[warmup + i]))
            ts.append(_time.perf_counter() - t0)
        return np.array(ts)

    run.timeit = timeit
    return run


_CACHE = {}


def kernel(**inputs) -> np.ndarray:
    in_maps, meta, bounds_g, bounds_n = _build_inputs(inputs)
    key = (tuple(int(v) for v in meta["slot_w"][:50]), meta["ntiles"],
           tuple(sorted(meta["eps"].items())), meta["head_b2"],
           meta["f_ncols"], meta["b_ncols"])
    if key not in _CACHE:
        nc = _build_kernel(meta)
        _CACHE[key] = _make_runner(nc)
    run = _CACHE[key]
    res = run(in_maps)
    out = np.zeros(G, np.float32)
    for c in range(NCORES):
        g0, g1 = bounds_g[c], bounds_g[c + 1]
        out[g0:g1] = res[c]["out"][0, :g1 - g0]
    return out
